# revision 3
# baseline (speedup 1.0000x reference)
"""Trainium2 Bass kernel for nn_Attention_Separate (8-core SPMD).

Sharding: head-parallel. Each of the 8 NeuronCores computes one attention
head end-to-end (Q/K/V projections, scores, softmax, AV), producing a
partial output out_h.T [D, B*N].  The head-sum becomes a ReduceScatter
over the 8 cores (chunked per 512-column query block so the collectives
overlap compute); each core ends with a [128, B*N] slice of the summed
output, gathered and transposed on the host.

All heavy matmuls run as float32r (full PE rate) or bf16; accumulation is
fp32 in PSUM.  Softmax needs no max-subtraction here: scores are ~N(0,
0.016) by construction, so exp() is well-conditioned.
"""

import sys

sys.path.insert(0, "/opt/trn_rl_repo")

import numpy as np

# Problem shapes (hardcoded per the contract).
B = 2
N = 2048
H = 8
R = 64
D = 1024
NTOK = B * N  # 4096
P = 128
KT = D // P  # 8 contraction tiles over embed dim
MT = N // P  # 16 key tiles per batch
NSB = 512  # query superblock (matmul free dim)
NBLK = N // NSB  # 4 query superblocks per batch
DBT = D // P  # 8 output-dim tiles
N_CORES = 8

_state: dict = {}


def _build_nc():
    import concourse.bacc as bacc
    import concourse.tile as tile
    from concourse import mybir

    f32 = mybir.dt.float32
    f32r = mybir.dt.float32r
    bf16 = mybir.dt.bfloat16
    Exp = mybir.ActivationFunctionType.Exp

    nc = bacc.Bacc(
        "TRN2", target_bir_lowering=False, debug=False, num_devices=N_CORES
    )
    xT = nc.dram_tensor("xT", [D, NTOK], f32, kind="ExternalInput").ap()
    wqT = nc.dram_tensor("wqT", [D, R], f32, kind="ExternalInput").ap()
    wkT = nc.dram_tensor("wkT", [D, R], f32, kind="ExternalInput").ap()
    wvT = nc.dram_tensor("wvT", [D, D], f32, kind="ExternalInput").ap()
    out_rs = nc.dram_tensor("out_rs", [P, NTOK], f32, kind="ExternalOutput").ap()

    xT_v = xT.bitcast(f32r).rearrange("(kt p) n -> kt p n", p=P)
    wqT_v = wqT.bitcast(f32r).rearrange("(kt p) r -> kt p r", p=P)
    wkT_v = wkT.bitcast(f32r).rearrange("(kt p) r -> kt p r", p=P)
    wvT_v = wvT.bitcast(f32r).rearrange("(kt p) d -> kt p d", p=P)

    with tile.TileContext(nc) as tc:
        with (
            tc.tile_pool(name="consts", bufs=1) as consts,
            tc.tile_pool(name="xtp", bufs=1) as xtp,
            tc.tile_pool(name="qkp", bufs=1) as qkp,
            tc.tile_pool(name="vpool", bufs=1) as vpool,
            tc.tile_pool(name="utp", bufs=2) as utp,
            tc.tile_pool(name="small", bufs=2) as small,
            tc.tile_pool(name="stage", bufs=3) as stagep,
            tc.tile_pool(name="proj_ps", bufs=2, space="PSUM") as proj_ps,
            tc.tile_pool(name="s_ps", bufs=2, space="PSUM") as s_ps,
            tc.tile_pool(name="av_ps", bufs=2, space="PSUM") as av_ps,
            tc.tile_pool(name="rs_ps", bufs=1, space="PSUM") as rs_ps,
            tc.tile_pool(name="dram", bufs=2, space="DRAM") as dram,
        ):
            ones_sb = consts.tile([P, P], bf16)
            nc.vector.memset(ones_sb, 1.0)
            wq_sb = consts.tile([P, KT, R], f32r)
            wk_sb = consts.tile([P, KT, R], f32r)
            wv_sb = consts.tile([P, KT, D], f32r)
            for k in range(KT):
                nc.sync.dma_start(out=wq_sb[:, k, :], in_=wqT_v[k])
                nc.sync.dma_start(out=wk_sb[:, k, :], in_=wkT_v[k])
                nc.sync.dma_start(out=wv_sb[:, k, :], in_=wvT_v[k])

            for b in range(B):
                # ---- load x.T for this batch ----
                xt = xtp.tile([P, KT, N], f32r, tag="xt")
                for k in range(KT):
                    nc.sync.dma_start(
                        out=xt[:, k, :], in_=xT_v[k, :, b * N : (b + 1) * N]
                    )
                # ---- Q/K projections -> [R, N] bf16 (transposed layout) ----
                qt = qkp.tile([R, N], bf16, tag="qt")
                kt_sb = qkp.tile([R, N], bf16, tag="kt")
                for nb in range(NBLK):
                    nsl = slice(nb * NSB, (nb + 1) * NSB)
                    qps = proj_ps.tile([P, NSB], f32, tag="proj")
                    for k in range(KT):
                        nc.tensor.matmul(
                            qps[:R, :],
                            wq_sb[:, k, :],
                            xt[:, k, nsl],
                            start=(k == 0),
                            stop=(k == KT - 1),
                        )
                    nc.vector.tensor_copy(qt[:, nsl], qps[:R, :])
                    kps = proj_ps.tile([P, NSB], f32, tag="proj")
                    for k in range(KT):
                        nc.tensor.matmul(
                            kps[:R, :],
                            wk_sb[:, k, :],
                            xt[:, k, nsl],
                            start=(k == 0),
                            stop=(k == KT - 1),
                        )
                    nc.vector.tensor_copy(kt_sb[:, nsl], kps[:R, :])
                # ---- V projection -> [m, d] bf16 (natural layout) ----
                v_sb = vpool.tile([P, MT, D], bf16, tag="v")
                for mt in range(MT):
                    for db in range(2):
                        vps = proj_ps.tile([P, NSB], f32, tag="proj")
                        for k in range(KT):
                            nc.tensor.matmul(
                                vps,
                                xt[:, k, mt * P : (mt + 1) * P],
                                wv_sb[:, k, db * NSB : (db + 1) * NSB],
                                start=(k == 0),
                                stop=(k == KT - 1),
                            )
                        nc.vector.tensor_copy(
                            v_sb[:, mt, db * NSB : (db + 1) * NSB], vps
                        )
                # ---- attention over query superblocks ----
                for ns in range(NBLK):
                    nsl = slice(ns * NSB, (ns + 1) * NSB)
                    ut = utp.tile([P, MT, NSB], bf16, tag="ut")
                    rs_psum = rs_ps.tile([P, NSB], f32, tag="rs")
                    for mt in range(MT):
                        sps = s_ps.tile([P, NSB], f32, tag="s")
                        # S.T[m-tile, n-block] = K_h Q_h.T  (bf16)
                        nc.tensor.matmul(
                            sps,
                            kt_sb[:, mt * P : (mt + 1) * P],
                            qt[:, nsl],
                            start=True,
                            stop=True,
                        )
                        nc.scalar.activation(ut[:, mt, :], sps, Exp)
                        # broadcast row-sum: every psum partition gets
                        # sum_m exp(S.T[m, n])
                        nc.tensor.matmul(
                            rs_psum,
                            ones_sb,
                            ut[:, mt, :],
                            start=(mt == 0),
                            stop=(mt == MT - 1),
                        )
                    rinv = small.tile([P, NSB], f32, tag="rinv")
                    nc.vector.reciprocal(rinv, rs_psum)
                    cc_in = dram.tile([D, NSB], f32, tag="ccin")
                    for dt_ in range(DBT):
                        avps = av_ps.tile([P, NSB], f32, tag="av")
                        for mt in range(MT):
                            nc.tensor.matmul(
                                avps,
                                v_sb[:, mt, dt_ * P : (dt_ + 1) * P],
                                ut[:, mt, :],
                                start=(mt == 0),
                                stop=(mt == MT - 1),
                            )
                        st = stagep.tile([P, NSB], f32, tag="stage")
                        nc.vector.tensor_mul(st, avps, rinv)
                        nc.sync.dma_start(
                            out=cc_in[dt_ * P : (dt_ + 1) * P, :], in_=st
                        )
                    cc_out = dram.tile([P, NSB], f32, tag="ccout")
                    nc.gpsimd.collective_compute(
                        "ReduceScatter",
                        mybir.AluOpType.add,
                        replica_groups=[list(range(N_CORES))],
                        ins=[cc_in.opt()],
                        outs=[cc_out.opt()],
                    )
                    nc.sync.dma_start(
                        out=out_rs[:, (b * NBLK + ns) * NSB : (b * NBLK + ns + 1) * NSB],
                        in_=cc_out,
                    )
    nc.compile()
    return nc


def _get_runner():
    """Build (once) a jitted 8-core SPMD callable for the bass module.

    Mirrors bass2jax.run_bass_via_pjrt but caches the jitted function so
    repeated calls don't re-trace/re-compile (needed for timing loops).
    """
    if "runner" in _state:
        return _state["runner"]

    import jax
    from jax.sharding import Mesh, PartitionSpec
    from jax.experimental.shard_map import shard_map
    from concourse import bass2jax, mybir

    bass2jax.install_neuronx_cc_hook()
    nc = _build_nc()

    in_names: list[str] = []
    out_names: list[str] = []
    out_avals = []
    zero_outs: list[np.ndarray] = []
    partition_name = (
        nc.partition_id_tensor.name if nc.partition_id_tensor else None
    )
    for alloc in nc.m.functions[0].allocations:
        if not isinstance(alloc, mybir.MemoryLocationSet):
            continue
        name = alloc.memorylocations[0].name
        if alloc.kind == "ExternalInput":
            if name != partition_name:
                in_names.append(name)
        elif alloc.kind == "ExternalOutput":
            shape = tuple(alloc.tensor_shape)
            dtype = mybir.dt.np(alloc.dtype)
            out_names.append(name)
            out_avals.append(jax.core.ShapedArray(shape, dtype))
            zero_outs.append(np.zeros(shape, dtype))
    n_params = len(in_names)
    n_outs = len(out_avals)
    all_in_names = in_names + out_names
    if partition_name is not None:
        all_in_names = all_in_names + [partition_name]

    def _body(*args):
        operands = list(args)
        if partition_name is not None:
            operands.append(bass2jax.partition_id_tensor())
        outs = bass2jax._bass_exec_p.bind(
            *operands,
            out_avals=tuple(out_avals),
            in_names=tuple(all_in_names),
            out_names=tuple(out_names),
            lowering_input_output_aliases=(),
            sim_require_finite=True,
            sim_require_nnan=True,
            nc=nc,
        )
        return tuple(outs)

    devices = jax.devices()[:N_CORES]
    assert len(devices) == N_CORES, f"need {N_CORES} cores, saw {len(jax.devices())}"
    mesh = Mesh(np.asarray(devices), ("core",))
    in_specs = (PartitionSpec("core"),) * (n_params + n_outs)
    out_specs = (PartitionSpec("core"),) * n_outs
    donate = tuple(range(n_params, n_params + n_outs))
    sharded = jax.jit(
        shard_map(
            _body, mesh=mesh, in_specs=in_specs, out_specs=out_specs, check_rep=False
        ),
        donate_argnums=donate,
        keep_unused=True,
    )

    def run(in_maps: list[dict[str, np.ndarray]]):
        concat_in = [
            np.concatenate([np.asarray(in_maps[c][nm]) for c in range(N_CORES)], axis=0)
            for nm in in_names
        ]
        concat_zeros = [
            np.zeros((N_CORES * z.shape[0], *z.shape[1:]), z.dtype) for z in zero_outs
        ]
        out_arrs = sharded(*concat_in, *concat_zeros)
        return [
            {
                nm: np.asarray(out_arrs[i]).reshape(N_CORES, *out_avals[i].shape)[c]
                for i, nm in enumerate(out_names)
            }
            for c in range(N_CORES)
        ]

    runner = {"run": run, "sharded": sharded, "in_names": in_names,
              "out_names": out_names, "out_avals": out_avals,
              "zero_outs": zero_outs, "mesh": mesh, "nc": nc}
    _state["runner"] = runner
    return runner


def _make_in_maps(x, Wq, Wk, Wv):
    xT = np.ascontiguousarray(
        x.reshape(NTOK, D).T.astype(np.float32)
    )  # [D, NTOK]
    in_maps = []
    for c in range(N_CORES):
        wq_c = np.ascontiguousarray(Wq[c * R : (c + 1) * R, :].T.astype(np.float32))
        wk_c = np.ascontiguousarray(Wk[c * R : (c + 1) * R, :].T.astype(np.float32))
        wv_c = np.ascontiguousarray(Wv[c * D : (c + 1) * D, :].T.astype(np.float32))
        in_maps.append({"xT": xT, "wqT": wq_c, "wkT": wk_c, "wvT": wv_c})
    return in_maps


def kernel(x, Wq, Wk, Wv):
    runner = _get_runner()
    results = runner["run"](_make_in_maps(x, Wq, Wk, Wv))
    outT = np.concatenate(
        [results[c]["out_rs"] for c in range(N_CORES)], axis=0
    )  # [D, NTOK]
    return np.ascontiguousarray(outT.T).reshape(B, N, D).astype(np.float32)


# revision 22
# speedup vs baseline: 85.6701x; 85.6701x over previous
"""Trainium2 Bass kernel for nn_Attention_Separate (8-core SPMD).

Sharding: output-dim sharded ("d-shard"). Core c computes out.T rows
[c*128, (c+1)*128) — i.e. a 128-wide slice of the embedding dim of the
output — for ALL heads and both batches. The head-sum stays core-local,
so there is NO cross-core reduction: the unshard is a pure concat +
transpose on the host.

Rationale: ncfw collectives in this axon runtime measure ~2.7ms for the
16.8MB head-sum reduce-scatter (and ~10ms when overlapped with kernel
traffic), dwarfing the ~250us of extra matmul that d-sharding costs
(every core redoes all heads' Q/K projections + scores + softmax, but
only its 1/8 of the V projection and of attn@V).

Per-core structure (all matmuls bf16 inputs, fp32 PSUM accumulate):
  - Q/K proj: 2 heads packed per matmul (out partitions 0-63 = head j,
    64-127 = head j+4) -> qt_all/kt_all [128, 4, 2048] per batch.
  - V proj: only this core's 128-wide d-slice of each head's V.
  - Scores S.T = K Q^T per head pair: the two K=64 matmuls of heads
    (j, j+4) target disjoint PE row-groups and run concurrently.
  - softmax: scores ~ N(0, 0.016), so exp() without max-subtraction is
    exact; row-sums accumulate on DVE in bulk [128, 2, 512] bf16 ops,
    then one ones-matmul broadcasts sum_m exp over all 128 partitions;
    normalization multiplies by the reciprocal AFTER attn@V.
  - attn@V accumulates unnormalized over m in PSUM; per-head normalize +
    head-sum accumulate on DVE.
"""

import sys

sys.path.insert(0, "/opt/trn_rl_repo")

import numpy as np

# Problem shapes (hardcoded per the contract).
B = 2
N = 2048
H = 8
R = 64
D = 1024
NTOK = B * N  # 4096
P = 128
KT = D // P  # 8 contraction tiles over embed dim
MT = N // P  # 16 key tiles per batch
NSB = 512  # query superblock (matmul free dim)
NBLK = N // NSB  # 4 query superblocks per batch
N_CORES = 8

_state: dict = {}


def _build_nc_dshard(rep=1):
    import concourse.bacc as bacc
    import concourse.tile as tile
    from concourse.tile_rust import add_dep_helper
    from concourse import mybir

    f32 = mybir.dt.float32
    bf16 = mybir.dt.bfloat16
    Exp = mybir.ActivationFunctionType.Exp

    nc = bacc.Bacc(
        "TRN2", target_bir_lowering=False, debug=False, num_devices=N_CORES
    )
    xtb = nc.dram_tensor("xtb", [D, NTOK], bf16, kind="ExternalInput").ap()
    wq_p = nc.dram_tensor("wq_p", [D, 4, P], bf16, kind="ExternalInput").ap()
    wk_p = nc.dram_tensor("wk_p", [D, 4, P], bf16, kind="ExternalInput").ap()
    wv_p = nc.dram_tensor("wv_p", [D, H * P], bf16, kind="ExternalInput").ap()
    out_dT = nc.dram_tensor("out_dT", [P, NTOK], f32, kind="ExternalOutput").ap()

    xtb_v = xtb.rearrange("(kt p) n -> kt p n", p=P)
    wq_v = wq_p.rearrange("(kt p) j m -> kt p j m", p=P)
    wk_v = wk_p.rearrange("(kt p) j m -> kt p j m", p=P)
    wv_v = wv_p.rearrange("(kt p) hd -> kt p hd", p=P)

    with tile.TileContext(nc) as tc:
        with (
            tc.tile_pool(name="consts", bufs=1) as consts,
            tc.tile_pool(name="xtp", bufs=1) as xtp,
            tc.tile_pool(name="qkp", bufs=1) as qkp,
            tc.tile_pool(name="vpool", bufs=1) as vpool,
            tc.tile_pool(name="utp", bufs=4) as utp,
            tc.tile_pool(name="accp", bufs=4) as accp,
            tc.tile_pool(name="small", bufs=2) as small,
            tc.tile_pool(name="outp", bufs=2) as outp,
            # PSUM budget (8 banks): s_ps 2x[128,1024]=4, av_ps 2,
            # rs_ps 2 (shared between projection psum and row-sum psum)
            tc.tile_pool(name="s_ps", bufs=2, space="PSUM") as s_ps,
            tc.tile_pool(name="av_ps", bufs=2, space="PSUM") as av_ps,
            tc.tile_pool(name="rs_ps", bufs=2, space="PSUM") as rs_ps,
        ):
            ones_sb = consts.tile([P, P], bf16)
            nc.vector.memset(ones_sb, 1.0)
            wq_sb = consts.tile([P, KT, 4, P], bf16)
            wk_sb = consts.tile([P, KT, 4, P], bf16)
            wv_sb = consts.tile([P, KT, H * P], bf16)
            for k in range(KT):
                nc.sync.dma_start(out=wq_sb[:, k], in_=wq_v[k])
                nc.sync.dma_start(out=wk_sb[:, k], in_=wk_v[k])
                nc.sync.dma_start(out=wv_sb[:, k], in_=wv_v[k])

            prev_rep_tail = None
            for _rep in range(rep):
                for b in range(B):
                    xt = xtp.tile([P, KT, N], bf16, tag="xt")
                    for k in range(KT):
                        ld = nc.sync.dma_start(
                            out=xt[:, k, :], in_=xtb_v[k, :, b * N : (b + 1) * N]
                        )
                        if prev_rep_tail is not None:
                            add_dep_helper(ld.ins, prev_rep_tail.ins,
                                           reason="serialize reps for timing")
                    # ---- Q/K projections, 2 heads per matmul ----
                    # qt_all[p, j, n]: p 0-63 = head j, p 64-127 = head j+4
                    qt_all = qkp.tile([P, 4, N], bf16, tag="qt")
                    kt_all = qkp.tile([P, 4, N], bf16, tag="kt")
                    for j in range(4):
                        for nb in range(NBLK):
                            nsl = slice(nb * NSB, (nb + 1) * NSB)
                            qps = rs_ps.tile([P, NSB], f32, tag="rsproj")
                            for k in range(KT):
                                nc.tensor.matmul(
                                    qps, wq_sb[:, k, j, :], xt[:, k, nsl],
                                    start=(k == 0), stop=(k == KT - 1),
                                )
                            nc.vector.tensor_copy(qt_all[:, j, nsl], qps)
                            kps = rs_ps.tile([P, NSB], f32, tag="rsproj")
                            for k in range(KT):
                                nc.tensor.matmul(
                                    kps, wk_sb[:, k, j, :], xt[:, k, nsl],
                                    start=(k == 0), stop=(k == KT - 1),
                                )
                            nc.vector.tensor_copy(kt_all[:, j, nsl], kps)
                    # ---- V projection for this core's d-slice, all heads ----
                    # v_sb[p, mt, h*128+dd]
                    v_sb = vpool.tile([P, MT, H * P], bf16, tag="v")
                    for mt in range(MT):
                        for hg in range(2):
                            hsl = slice(hg * 4 * P, (hg + 1) * 4 * P)
                            vps = rs_ps.tile([P, NSB], f32, tag="rsproj")
                            for k in range(KT):
                                nc.tensor.matmul(
                                    vps,
                                    xt[:, k, mt * P : (mt + 1) * P],
                                    wv_sb[:, k, hsl],
                                    start=(k == 0), stop=(k == KT - 1),
                                )
                            nc.vector.tensor_copy(v_sb[:, mt, hsl], vps)
                    # ---- attention: head pairs (j, j+4) in disjoint PE
                    # row-groups so their K=64 score matmuls overlap ----
                    for ns in range(NBLK):
                        nsl = slice(ns * NSB, (ns + 1) * NSB)
                        acc_out = outp.tile([P, NSB], f32, tag="accout")
                        for jj in range(4):
                            h0, h1 = jj, jj + 4
                            q0 = qt_all[0:64, jj, nsl]
                            q1 = qt_all[64:128, jj, nsl]
                            avps0 = av_ps.tile([P, NSB], f32, tag="av")
                            avps1 = av_ps.tile([P, NSB], f32, tag="av")
                            acc_big = accp.tile([P, 2, NSB], bf16, tag="acc")
                            for mt in range(MT):
                                msl = slice(mt * P, (mt + 1) * P)
                                sbig = s_ps.tile([P, 2, NSB], f32, tag="s")
                                nc.tensor.matmul(
                                    sbig[:, 0, :], kt_all[0:64, jj, msl], q0,
                                    start=True, stop=True,
                                )
                                nc.tensor.matmul(
                                    sbig[:, 1, :], kt_all[64:128, jj, msl], q1,
                                    start=True, stop=True,
                                )
                                utb = utp.tile([P, 2, NSB], bf16, tag="ut")
                                nc.scalar.activation(utb, sbig, Exp)
                                # bulk row-sum partials on DVE: slice 0
                                # accumulates h0, slice 1 accumulates h1
                                if mt == 0:
                                    nc.vector.tensor_copy(acc_big, utb)
                                else:
                                    nc.vector.tensor_add(acc_big, acc_big, utb)
                                nc.tensor.matmul(
                                    avps0, v_sb[:, mt, h0 * P : (h0 + 1) * P],
                                    utb[:, 0, :],
                                    start=(mt == 0), stop=(mt == MT - 1),
                                )
                                nc.tensor.matmul(
                                    avps1, v_sb[:, mt, h1 * P : (h1 + 1) * P],
                                    utb[:, 1, :],
                                    start=(mt == 0), stop=(mt == MT - 1),
                                )
                            for hi, avps in enumerate([avps0, avps1]):
                                rs_psum = rs_ps.tile([P, NSB], f32, tag="rsproj")
                                nc.tensor.matmul(rs_psum, ones_sb,
                                                 acc_big[:, hi, :],
                                                 start=True, stop=True)
                                rinv = small.tile([P, NSB], f32, tag="rinv")
                                nc.vector.reciprocal(rinv, rs_psum)
                                if jj == 0 and hi == 0:
                                    nc.vector.tensor_mul(acc_out, avps, rinv)
                                else:
                                    tmp = small.tile([P, NSB], f32, tag="tmp")
                                    nc.vector.tensor_mul(tmp, avps, rinv)
                                    nc.vector.tensor_add(acc_out, acc_out, tmp)
                        prev_rep_tail = nc.sync.dma_start(
                            out=out_dT[:, b * N + ns * NSB : b * N + (ns + 1) * NSB],
                            in_=acc_out,
                        )
    nc.compile()
    return nc


def _get_runner(mode="dshard"):
    """Build (once per mode) a jitted 8-core SPMD callable for the bass
    module. Mirrors bass2jax.run_bass_via_pjrt but caches the jitted
    function so repeated calls don't re-trace/re-compile."""
    rep = 1
    if "@" in mode:
        mode, rep_s = mode.split("@")
        rep = int(rep_s)
    key = f"runner_{mode}@{rep}"
    if key in _state:
        return _state[key]

    import jax
    from jax.sharding import Mesh, PartitionSpec
    from jax.experimental.shard_map import shard_map
    from concourse import bass2jax, mybir

    bass2jax.install_neuronx_cc_hook()
    assert mode == "dshard", f"unknown mode {mode}"
    nc = _build_nc_dshard(rep=rep)

    in_names: list[str] = []
    out_names: list[str] = []
    out_avals = []
    zero_outs: list[np.ndarray] = []
    partition_name = (
        nc.partition_id_tensor.name if nc.partition_id_tensor else None
    )
    for alloc in nc.m.functions[0].allocations:
        if not isinstance(alloc, mybir.MemoryLocationSet):
            continue
        name = alloc.memorylocations[0].name
        if alloc.kind == "ExternalInput":
            if name != partition_name:
                in_names.append(name)
        elif alloc.kind == "ExternalOutput":
            shape = tuple(alloc.tensor_shape)
            dtype = mybir.dt.np(alloc.dtype)
            out_names.append(name)
            out_avals.append(jax.core.ShapedArray(shape, dtype))
            zero_outs.append(np.zeros(shape, dtype))
    n_params = len(in_names)
    n_outs = len(out_avals)
    all_in_names = in_names + out_names
    if partition_name is not None:
        all_in_names = all_in_names + [partition_name]

    def _body(*args):
        operands = list(args)
        if partition_name is not None:
            operands.append(bass2jax.partition_id_tensor())
        outs = bass2jax._bass_exec_p.bind(
            *operands,
            out_avals=tuple(out_avals),
            in_names=tuple(all_in_names),
            out_names=tuple(out_names),
            lowering_input_output_aliases=(),
            sim_require_finite=True,
            sim_require_nnan=True,
            nc=nc,
        )
        return tuple(outs)

    devices = jax.devices()[:N_CORES]
    assert len(devices) == N_CORES, f"need {N_CORES} cores, saw {len(jax.devices())}"
    mesh = Mesh(np.asarray(devices), ("core",))
    in_specs = (PartitionSpec("core"),) * (n_params + n_outs)
    out_specs = (PartitionSpec("core"),) * n_outs
    donate = tuple(range(n_params, n_params + n_outs))
    sharded = jax.jit(
        shard_map(
            _body, mesh=mesh, in_specs=in_specs, out_specs=out_specs, check_rep=False
        ),
        donate_argnums=donate,
        keep_unused=True,
    )

    def run(in_maps):
        concat_in = [
            np.concatenate([np.asarray(in_maps[c][nm]) for c in range(N_CORES)], axis=0)
            for nm in in_names
        ]
        concat_zeros = [
            np.zeros((N_CORES * z.shape[0], *z.shape[1:]), z.dtype) for z in zero_outs
        ]
        out_arrs = sharded(*concat_in, *concat_zeros)
        return [
            {
                nm: np.asarray(out_arrs[i]).reshape(N_CORES, *out_avals[i].shape)[c]
                for i, nm in enumerate(out_names)
            }
            for c in range(N_CORES)
        ]

    runner = {"run": run, "sharded": sharded, "in_names": in_names,
              "out_names": out_names, "out_avals": out_avals,
              "zero_outs": zero_outs, "mesh": mesh, "nc": nc}
    _state[key] = runner
    return runner


def _make_in_maps_dshard(x, Wq, Wk, Wv):
    import ml_dtypes

    bf16 = ml_dtypes.bfloat16
    xtb = np.ascontiguousarray(x.reshape(NTOK, D).T).astype(bf16)  # [D, NTOK]
    # wq_p[d, j, m]: m<64 -> head j, r=m ; m>=64 -> head j+4, r=m-64
    wq_p = np.empty((D, 4, P), dtype=bf16)
    wk_p = np.empty((D, 4, P), dtype=bf16)
    for j in range(4):
        wq_p[:, j, :64] = Wq[j * R : (j + 1) * R, :].T
        wq_p[:, j, 64:] = Wq[(j + 4) * R : (j + 5) * R, :].T
        wk_p[:, j, :64] = Wk[j * R : (j + 1) * R, :].T
        wk_p[:, j, 64:] = Wk[(j + 4) * R : (j + 5) * R, :].T
    in_maps = []
    for c in range(N_CORES):
        # wv_p[d_in, h*128+dd] = Wv[h*D + c*128 + dd, d_in]
        rows = np.concatenate(
            [np.arange(h * D + c * P, h * D + (c + 1) * P) for h in range(H)]
        )
        wv_c = np.ascontiguousarray(Wv[rows, :].T).astype(bf16)  # [D, H*P]
        in_maps.append({"xtb": xtb, "wq_p": wq_p, "wk_p": wk_p, "wv_p": wv_c})
    return in_maps


# kept for compatibility with test harnesses
_make_in_maps = _make_in_maps_dshard


def kernel(x, Wq, Wk, Wv, mode="dshard"):
    runner = _get_runner(mode)
    results = runner["run"](_make_in_maps_dshard(x, Wq, Wk, Wv))
    outT = np.concatenate(
        [results[c]["out_dT"] for c in range(N_CORES)], axis=0
    )  # [D, NTOK]
    return np.ascontiguousarray(outT.T).reshape(B, N, D).astype(np.float32)


# revision 24
# speedup vs baseline: 191.7881x; 2.2387x over previous
"""Trainium2 Bass kernel for nn_Attention_Separate (8-core SPMD).

Sharding: output-dim sharded ("d-shard"). Core c computes out.T rows
[c*128, (c+1)*128) — i.e. a 128-wide slice of the embedding dim of the
output — for ALL heads and both batches. The head-sum stays core-local,
so there is NO cross-core reduction: the unshard is a pure concat +
transpose on the host.

Rationale: ncfw collectives in this axon runtime measure ~2.7ms for the
16.8MB head-sum reduce-scatter (and ~10ms when overlapped with kernel
traffic), dwarfing the ~250us of extra matmul that d-sharding costs
(every core redoes all heads' Q/K projections + scores + softmax, but
only its 1/8 of the V projection and of attn@V).

Per-core structure (all matmuls bf16 inputs, fp32 PSUM accumulate):
  - Q/K proj: 2 heads packed per matmul (out partitions 0-63 = head j,
    64-127 = head j+4) -> qt_all/kt_all [128, 4, 2048] per batch.
  - V proj: only this core's 128-wide d-slice of each head's V.
  - Scores S.T = K Q^T per head pair: the two K=64 matmuls of heads
    (j, j+4) target disjoint PE row-groups and run concurrently.
  - softmax: scores ~ N(0, 0.016), so exp() without max-subtraction is
    exact; row-sums accumulate on DVE in bulk [128, 2, 512] bf16 ops,
    then one ones-matmul broadcasts sum_m exp over all 128 partitions;
    normalization multiplies by the reciprocal AFTER attn@V.
  - attn@V accumulates unnormalized over m in PSUM; per-head normalize +
    head-sum accumulate on DVE.
"""

import sys

sys.path.insert(0, "/opt/trn_rl_repo")

import numpy as np

# Problem shapes (hardcoded per the contract).
B = 2
N = 2048
H = 8
R = 64
D = 1024
NTOK = B * N  # 4096
P = 128
KT = D // P  # 8 contraction tiles over embed dim
MT = N // P  # 16 key tiles per batch
NSB = 512  # query superblock (matmul free dim)
NBLK = N // NSB  # 4 query superblocks per batch
N_CORES = 8

_state: dict = {}


def _build_nc_dshard(rep=1):
    import concourse.bacc as bacc
    import concourse.tile as tile
    from concourse.tile_rust import add_dep_helper
    from concourse import mybir

    f32 = mybir.dt.float32
    bf16 = mybir.dt.bfloat16
    Exp = mybir.ActivationFunctionType.Exp

    nc = bacc.Bacc(
        "TRN2", target_bir_lowering=False, debug=False, num_devices=N_CORES
    )
    xtb = nc.dram_tensor("xtb", [D, NTOK], bf16, kind="ExternalInput").ap()
    wq_p = nc.dram_tensor("wq_p", [D, 4, P], bf16, kind="ExternalInput").ap()
    wk_p = nc.dram_tensor("wk_p", [D, 4, P], bf16, kind="ExternalInput").ap()
    wv_p = nc.dram_tensor("wv_p", [D, H * P], bf16, kind="ExternalInput").ap()
    out_dT = nc.dram_tensor("out_dT", [P, NTOK], f32, kind="ExternalOutput").ap()

    xtb_v = xtb.rearrange("(kt p) n -> kt p n", p=P)
    wq_v = wq_p.rearrange("(kt p) j m -> kt p j m", p=P)
    wk_v = wk_p.rearrange("(kt p) j m -> kt p j m", p=P)
    wv_v = wv_p.rearrange("(kt p) hd -> kt p hd", p=P)

    with tile.TileContext(nc) as tc:
        with (
            tc.tile_pool(name="consts", bufs=1) as consts,
            tc.tile_pool(name="xtp", bufs=1) as xtp,
            tc.tile_pool(name="qkp", bufs=1) as qkp,
            tc.tile_pool(name="vpool", bufs=1) as vpool,
            tc.tile_pool(name="utp", bufs=4) as utp,
            tc.tile_pool(name="accp", bufs=4) as accp,
            tc.tile_pool(name="small", bufs=2) as small,
            tc.tile_pool(name="outp", bufs=2) as outp,
            # PSUM budget (8 banks): s_ps 2x[128,1024]=4, av_ps 2,
            # rs_ps 2 (shared between projection psum and row-sum psum)
            tc.tile_pool(name="s_ps", bufs=2, space="PSUM") as s_ps,
            tc.tile_pool(name="av_ps", bufs=2, space="PSUM") as av_ps,
            tc.tile_pool(name="rs_ps", bufs=2, space="PSUM") as rs_ps,
        ):
            ones_sb = consts.tile([P, P], bf16)
            nc.vector.memset(ones_sb, 1.0)
            wq_sb = consts.tile([P, KT, 4, P], bf16)
            wk_sb = consts.tile([P, KT, 4, P], bf16)
            wv_sb = consts.tile([P, KT, H * P], bf16)
            for k in range(KT):
                nc.sync.dma_start(out=wq_sb[:, k], in_=wq_v[k])
                nc.sync.dma_start(out=wk_sb[:, k], in_=wk_v[k])
                nc.sync.dma_start(out=wv_sb[:, k], in_=wv_v[k])

            prev_rep_tail = None
            for _rep in range(rep):
                for b in range(B):
                    xt = xtp.tile([P, KT, N], bf16, tag="xt")
                    for k in range(KT):
                        ld = nc.sync.dma_start(
                            out=xt[:, k, :], in_=xtb_v[k, :, b * N : (b + 1) * N]
                        )
                        if prev_rep_tail is not None:
                            add_dep_helper(ld.ins, prev_rep_tail.ins,
                                           reason="serialize reps for timing")
                    # ---- Q/K projections, 2 heads per matmul ----
                    # qt_all[p, j, n]: p 0-63 = head j, p 64-127 = head j+4
                    qt_all = qkp.tile([P, 4, N], bf16, tag="qt")
                    kt_all = qkp.tile([P, 4, N], bf16, tag="kt")
                    for j in range(4):
                        for nb in range(NBLK):
                            nsl = slice(nb * NSB, (nb + 1) * NSB)
                            qps = rs_ps.tile([P, NSB], f32, tag="rsproj")
                            for k in range(KT):
                                nc.tensor.matmul(
                                    qps, wq_sb[:, k, j, :], xt[:, k, nsl],
                                    start=(k == 0), stop=(k == KT - 1),
                                )
                            nc.vector.tensor_copy(qt_all[:, j, nsl], qps)
                            kps = rs_ps.tile([P, NSB], f32, tag="rsproj")
                            for k in range(KT):
                                nc.tensor.matmul(
                                    kps, wk_sb[:, k, j, :], xt[:, k, nsl],
                                    start=(k == 0), stop=(k == KT - 1),
                                )
                            nc.vector.tensor_copy(kt_all[:, j, nsl], kps)
                    # ---- V projection for this core's d-slice, all heads ----
                    # v_sb[p, mt, h*128+dd]
                    v_sb = vpool.tile([P, MT, H * P], bf16, tag="v")
                    for mt in range(MT):
                        for hg in range(2):
                            hsl = slice(hg * 4 * P, (hg + 1) * 4 * P)
                            vps = rs_ps.tile([P, NSB], f32, tag="rsproj")
                            for k in range(KT):
                                nc.tensor.matmul(
                                    vps,
                                    xt[:, k, mt * P : (mt + 1) * P],
                                    wv_sb[:, k, hsl],
                                    start=(k == 0), stop=(k == KT - 1),
                                )
                            nc.vector.tensor_copy(v_sb[:, mt, hsl], vps)
                    # ---- attention: head pairs (j, j+4) in disjoint PE
                    # row-groups so their K=64 score matmuls overlap ----
                    for ns in range(NBLK):
                        nsl = slice(ns * NSB, (ns + 1) * NSB)
                        acc_out = outp.tile([P, NSB], f32, tag="accout")
                        for jj in range(4):
                            h0, h1 = jj, jj + 4
                            q0 = qt_all[0:64, jj, nsl]
                            q1 = qt_all[64:128, jj, nsl]
                            avps0 = av_ps.tile([P, NSB], f32, tag="av")
                            avps1 = av_ps.tile([P, NSB], f32, tag="av")
                            acc_big = accp.tile([P, 2, NSB], bf16, tag="acc")
                            for mt in range(MT):
                                msl = slice(mt * P, (mt + 1) * P)
                                sbig = s_ps.tile([P, 2, NSB], f32, tag="s")
                                nc.tensor.matmul(
                                    sbig[:, 0, :], kt_all[0:64, jj, msl], q0,
                                    start=True, stop=True,
                                )
                                nc.tensor.matmul(
                                    sbig[:, 1, :], kt_all[64:128, jj, msl], q1,
                                    start=True, stop=True,
                                )
                                utb = utp.tile([P, 2, NSB], bf16, tag="ut")
                                nc.scalar.activation(utb, sbig, Exp)
                                # bulk row-sum partials on DVE: slice 0
                                # accumulates h0, slice 1 accumulates h1
                                if mt == 0:
                                    nc.vector.tensor_copy(acc_big, utb)
                                else:
                                    nc.vector.tensor_add(acc_big, acc_big, utb)
                                nc.tensor.matmul(
                                    avps0, v_sb[:, mt, h0 * P : (h0 + 1) * P],
                                    utb[:, 0, :],
                                    start=(mt == 0), stop=(mt == MT - 1),
                                )
                                nc.tensor.matmul(
                                    avps1, v_sb[:, mt, h1 * P : (h1 + 1) * P],
                                    utb[:, 1, :],
                                    start=(mt == 0), stop=(mt == MT - 1),
                                )
                            for hi, avps in enumerate([avps0, avps1]):
                                rs_psum = rs_ps.tile([P, NSB], f32, tag="rsproj")
                                nc.tensor.matmul(rs_psum, ones_sb,
                                                 acc_big[:, hi, :],
                                                 start=True, stop=True)
                                rinv = small.tile([P, NSB], f32, tag="rinv")
                                nc.vector.reciprocal(rinv, rs_psum)
                                if jj == 0 and hi == 0:
                                    nc.vector.tensor_mul(acc_out, avps, rinv)
                                else:
                                    tmp = small.tile([P, NSB], f32, tag="tmp")
                                    nc.vector.tensor_mul(tmp, avps, rinv)
                                    nc.vector.tensor_add(acc_out, acc_out, tmp)
                        prev_rep_tail = nc.sync.dma_start(
                            out=out_dT[:, b * N + ns * NSB : b * N + (ns + 1) * NSB],
                            in_=acc_out,
                        )
    nc.compile()
    return nc


def _get_runner(mode="dshard"):
    """Build (once per mode) a jitted 8-core SPMD callable for the bass
    module. Mirrors bass2jax.run_bass_via_pjrt but caches the jitted
    function so repeated calls don't re-trace/re-compile."""
    rep = 1
    if "@" in mode:
        mode, rep_s = mode.split("@")
        rep = int(rep_s)
    key = f"runner_{mode}@{rep}"
    if key in _state:
        return _state[key]

    import jax
    from jax.sharding import Mesh, PartitionSpec
    from jax.experimental.shard_map import shard_map
    from concourse import bass2jax, mybir

    bass2jax.install_neuronx_cc_hook()
    assert mode == "dshard", f"unknown mode {mode}"
    nc = _build_nc_dshard(rep=rep)

    in_names: list[str] = []
    out_names: list[str] = []
    out_avals = []
    zero_outs: list[np.ndarray] = []
    partition_name = (
        nc.partition_id_tensor.name if nc.partition_id_tensor else None
    )
    for alloc in nc.m.functions[0].allocations:
        if not isinstance(alloc, mybir.MemoryLocationSet):
            continue
        name = alloc.memorylocations[0].name
        if alloc.kind == "ExternalInput":
            if name != partition_name:
                in_names.append(name)
        elif alloc.kind == "ExternalOutput":
            shape = tuple(alloc.tensor_shape)
            dtype = mybir.dt.np(alloc.dtype)
            out_names.append(name)
            out_avals.append(jax.core.ShapedArray(shape, dtype))
            zero_outs.append(np.zeros(shape, dtype))
    n_params = len(in_names)
    n_outs = len(out_avals)
    all_in_names = in_names + out_names
    if partition_name is not None:
        all_in_names = all_in_names + [partition_name]

    def _body(*args):
        operands = list(args)
        if partition_name is not None:
            operands.append(bass2jax.partition_id_tensor())
        outs = bass2jax._bass_exec_p.bind(
            *operands,
            out_avals=tuple(out_avals),
            in_names=tuple(all_in_names),
            out_names=tuple(out_names),
            lowering_input_output_aliases=(),
            sim_require_finite=True,
            sim_require_nnan=True,
            nc=nc,
        )
        return tuple(outs)

    devices = jax.devices()[:N_CORES]
    assert len(devices) == N_CORES, f"need {N_CORES} cores, saw {len(jax.devices())}"
    mesh = Mesh(np.asarray(devices), ("core",))
    in_specs = (PartitionSpec("core"),) * (n_params + n_outs)
    out_specs = (PartitionSpec("core"),) * n_outs
    donate = tuple(range(n_params, n_params + n_outs))
    sharded = jax.jit(
        shard_map(
            _body, mesh=mesh, in_specs=in_specs, out_specs=out_specs, check_rep=False
        ),
        donate_argnums=donate,
        keep_unused=True,
    )

    def run(in_maps):
        concat_in = [
            np.concatenate([np.asarray(in_maps[c][nm]) for c in range(N_CORES)], axis=0)
            for nm in in_names
        ]
        concat_zeros = [
            np.zeros((N_CORES * z.shape[0], *z.shape[1:]), z.dtype) for z in zero_outs
        ]
        out_arrs = sharded(*concat_in, *concat_zeros)
        return [
            {
                nm: np.asarray(out_arrs[i]).reshape(N_CORES, *out_avals[i].shape)[c]
                for i, nm in enumerate(out_names)
            }
            for c in range(N_CORES)
        ]

    runner = {"run": run, "sharded": sharded, "in_names": in_names,
              "out_names": out_names, "out_avals": out_avals,
              "zero_outs": zero_outs, "mesh": mesh, "nc": nc}
    _state[key] = runner
    return runner


def _make_in_maps_dshard(x, Wq, Wk, Wv):
    import ml_dtypes

    bf16 = ml_dtypes.bfloat16
    xtb = np.ascontiguousarray(x.reshape(NTOK, D).T).astype(bf16)  # [D, NTOK]
    # wq_p[d, j, m]: m<64 -> head j, r=m ; m>=64 -> head j+4, r=m-64
    wq_p = np.empty((D, 4, P), dtype=bf16)
    wk_p = np.empty((D, 4, P), dtype=bf16)
    for j in range(4):
        wq_p[:, j, :64] = Wq[j * R : (j + 1) * R, :].T
        wq_p[:, j, 64:] = Wq[(j + 4) * R : (j + 5) * R, :].T
        wk_p[:, j, :64] = Wk[j * R : (j + 1) * R, :].T
        wk_p[:, j, 64:] = Wk[(j + 4) * R : (j + 5) * R, :].T
    in_maps = []
    for c in range(N_CORES):
        # wv_p[d_in, h*128+dd] = Wv[h*D + c*128 + dd, d_in]
        rows = np.concatenate(
            [np.arange(h * D + c * P, h * D + (c + 1) * P) for h in range(H)]
        )
        wv_c = np.ascontiguousarray(Wv[rows, :].T).astype(bf16)  # [D, H*P]
        in_maps.append({"xtb": xtb, "wq_p": wq_p, "wk_p": wk_p, "wv_p": wv_c})
    return in_maps


# kept for compatibility with test harnesses
_make_in_maps = _make_in_maps_dshard


def kernel(x, Wq, Wk, Wv, mode="dshard"):
    runner = _get_runner(mode)
    results = runner["run"](_make_in_maps_dshard(x, Wq, Wk, Wv))
    outT = np.concatenate(
        [results[c]["out_dT"] for c in range(N_CORES)], axis=0
    )  # [D, NTOK]
    return np.ascontiguousarray(outT.T).reshape(B, N, D).astype(np.float32)
